# revision 9
# baseline (speedup 1.0000x reference)
"""Trainium2 Bass kernel for nn_AttentionBlock (GroupNorm + single-head
self-attention over 64x64 spatial positions + projection + residual).

Sharding: data-parallel over batch. 8 batch elements -> 8 NeuronCores.
Each core runs an identical program on its own batch element; weights are
replicated. No collectives.

Host-side algebraic folds (exact):
  - 1/sqrt(C) softmax scale and bq folded into the Q path (wq/16, bq/16).
  - bk dropped: adds a per-query constant to logits, cancels in softmax.
  - wp folded into V: wvp = wp @ wv, bvp = wp @ bv + bp. The attention
    matmul then directly produces the projected output (saves a whole
    [256x256]x[256x4096] matmul per core), and since softmax rows sum to 1
    the combined bias is added per-key to vp before attention.

Device-side layout (per core):
  x, xn, q, k stored [c(2x128 part), n=4096 free]; scores computed
  transposed  sT[j, i] (j on partitions) so softmax denominators come out
  of the attention matmul itself via an appended ones-column on vpT.
  exp() without max subtraction (logits ~ +-3, fp32/bf16 safe).
  All big matmuls in bf16 (1 cyc/row on PE), fp32 PSUM accumulation;
  residual added in fp32.
"""

import numpy as np
import ml_dtypes

import concourse.bass as bass
import concourse.mybir as mybir
from concourse import bacc, tile
from concourse.bass_utils import run_bass_kernel_spmd

B, C, H, W = 8, 256, 64, 64
HW = H * W           # 4096 positions
G = 8                # groups
GS = C // G          # 32 channels per group
EPS = 1e-5
NCORES = 8
CC = 2               # channel chunks of 128
JC = HW // 128       # 32 key chunks
IB = HW // 512       # 8 query blocks of 512
BF16 = ml_dtypes.bfloat16

f32 = mybir.dt.float32
bf16 = mybir.dt.bfloat16
AF = mybir.ActivationFunctionType
AX = mybir.AxisListType


def build_program(nc: bass.Bass):
    """Emit the per-core program (SPMD: same program on all 8 cores)."""
    x_d = nc.dram_tensor("x", [C, HW], f32, kind="ExternalInput").ap()
    wqT_d = nc.dram_tensor("wqT", [C, C], bf16, kind="ExternalInput").ap()
    wkT_d = nc.dram_tensor("wkT", [C, C], bf16, kind="ExternalInput").ap()
    wvpT_d = nc.dram_tensor("wvpT", [C, C], bf16, kind="ExternalInput").ap()
    bq_d = nc.dram_tensor("bq", [C, 1], f32, kind="ExternalInput").ap()
    bvpb_d = nc.dram_tensor("bvpb", [128, C], bf16, kind="ExternalInput").ap()
    gam_d = nc.dram_tensor("gam", [C, 1], f32, kind="ExternalInput").ap()
    bet_d = nc.dram_tensor("bet", [C, 1], f32, kind="ExternalInput").ap()
    gsum_d = nc.dram_tensor("gsum", [C, G], f32, kind="ExternalInput").ap()
    gbc_d = nc.dram_tensor("gbc", [G, C], f32, kind="ExternalInput").ap()
    ident_d = nc.dram_tensor("ident", [128, 128], bf16, kind="ExternalInput").ap()
    out_d = nc.dram_tensor("out", [C, HW], f32, kind="ExternalOutput").ap()

    with tile.TileContext(nc) as tc:
        _body(tc, x_d, wqT_d, wkT_d, wvpT_d, bq_d, bvpb_d, gam_d, bet_d,
              gsum_d, gbc_d, ident_d, out_d)
    nc.compile()
    return nc


def _body(tc, x_d, wqT_d, wkT_d, wvpT_d, bq_d, bvpb_d, gam_d, bet_d,
          gsum_d, gbc_d, ident_d, out_d):
    nc = tc.nc
    from contextlib import ExitStack

    with ExitStack() as ctx:
        const = ctx.enter_context(tc.tile_pool(name="const", bufs=1))
        persist = ctx.enter_context(tc.tile_pool(name="persist", bufs=1))

        # ---- constants / weights to SBUF ----
        wqT_t = const.tile([128, CC, C], bf16)
        wkT_t = const.tile([128, CC, C], bf16)
        wvpT_t = const.tile([128, CC, C], bf16)
        bq_t = const.tile([128, CC, 1], f32)
        gam_t = const.tile([128, CC, 1], f32)
        bet_t = const.tile([128, CC, 1], f32)
        gsum_t = const.tile([128, CC, G], f32)
        bvpb_t = const.tile([128, C], bf16)
        gbc_t = const.tile([G, C], f32)
        ident_t = const.tile([128, 128], bf16)
        zc_t = const.tile([128, 1], f32)
        eps_t = const.tile([G, 1], f32)
        nc.vector.memset(zc_t[:], 0.0)
        nc.vector.memset(eps_t[:], EPS)
        # activation() with a float bias resolves through this registry
        nc.const_aps.aps[(f32, 0.0)] = zc_t[:]
        for cc in range(CC):
            r = slice(cc * 128, (cc + 1) * 128)
            nc.sync.dma_start(wqT_t[:, cc, :], wqT_d[r, :])
            nc.sync.dma_start(wkT_t[:, cc, :], wkT_d[r, :])
            nc.sync.dma_start(wvpT_t[:, cc, :], wvpT_d[r, :])
            nc.sync.dma_start(bq_t[:, cc, :], bq_d[r, :])
            nc.sync.dma_start(gam_t[:, cc, :], gam_d[r, :])
            nc.sync.dma_start(bet_t[:, cc, :], bet_d[r, :])
            nc.sync.dma_start(gsum_t[:, cc, :], gsum_d[r, :])
        nc.sync.dma_start(bvpb_t[:], bvpb_d[:])
        nc.sync.dma_start(gbc_t[:], gbc_d[:])
        nc.sync.dma_start(ident_t[:], ident_d[:])

        # ---- x to SBUF (one DMA per chunk; a single InstDMACopy already
        # fans out across all 16 SDMA engines) ----
        x_t = persist.tile([128, CC, HW], f32)
        for cc in range(CC):
            nc.sync.dma_start(x_t[:, cc, :], x_d[cc * 128:(cc + 1) * 128, :])

        xn_t = persist.tile([128, CC, HW], bf16)
        q_t = persist.tile([128, CC, HW], bf16)
        k_t = persist.tile([128, CC, HW], bf16)
        vpT_t = persist.tile([128, JC, C + 1], bf16)
        o2_t = persist.tile([128, HW // 128, C], bf16)

        # ===================== GroupNorm =====================
        with tc.tile_pool(name="gn_ps", bufs=1, space="PSUM") as gn_ps, \
             tc.tile_pool(name="gn_sc", bufs=2) as gn_sc, \
             tc.tile_pool(name="stats", bufs=1) as stats_p:
            stat_t = stats_p.tile([128, CC, 2], f32)   # (sum, sumsq) per channel
            for cc in range(CC):
                sq_t = gn_sc.tile([128, HW], bf16)
                nc.vector.reduce_sum(stat_t[:, cc, 0:1], x_t[:, cc, :], axis=AX.X)
                nc.scalar.activation(sq_t[:], x_t[:, cc, :], AF.Square,
                                     accum_out=stat_t[:, cc, 1:2])
            gstat_ps = gn_ps.tile([G, 2], f32)
            for cc in range(CC):
                nc.tensor.matmul(gstat_ps[:], lhsT=gsum_t[:, cc, :],
                                 rhs=stat_t[:, cc, :],
                                 start=(cc == 0), stop=(cc == 1))
            ms_t = stats_p.tile([G, 4], f32)   # mean, Ex2, mean^2, var
            mr_t = stats_p.tile([G, 2], f32)   # mean, rstd
            lv_t = stats_p.tile([G, 1], f32)
            inv_n = 1.0 / float(GS * HW)
            nc.vector.tensor_scalar_mul(ms_t[:, 0:2], gstat_ps[:, 0:2], inv_n)
            nc.vector.tensor_copy(mr_t[:, 0:1], ms_t[:, 0:1])
            nc.vector.tensor_mul(ms_t[:, 2:3], ms_t[:, 0:1], ms_t[:, 0:1])
            nc.vector.tensor_sub(ms_t[:, 3:4], ms_t[:, 1:2], ms_t[:, 2:3])
            # rstd = exp(-0.5 * log(var + eps)); Log/Exp share one ACT table set
            nc.scalar.activation(lv_t[:], ms_t[:, 3:4], AF.Ln, bias=eps_t[:])
            nc.scalar.activation(mr_t[:, 1:2], lv_t[:], AF.Exp, scale=-0.5)

            ab_t = stats_p.tile([128, CC, 2], f32)   # A=rstd*gamma, B=beta-mean*A
            tmp_t = stats_p.tile([128, 1], f32)
            for cc in range(CC):
                bc_ps = gn_ps.tile([128, 2], f32, tag="bc", name="bc_ps")
                nc.tensor.matmul(bc_ps[:], lhsT=gbc_t[:, cc * 128:(cc + 1) * 128],
                                 rhs=mr_t[:], start=True, stop=True)
                nc.vector.tensor_mul(ab_t[:, cc, 0:1], bc_ps[:, 1:2], gam_t[:, cc, :])
                nc.vector.tensor_mul(tmp_t[:], bc_ps[:, 0:1], ab_t[:, cc, 0:1])
                nc.vector.tensor_sub(ab_t[:, cc, 1:2], bet_t[:, cc, :], tmp_t[:])
                nc.scalar.activation(xn_t[:, cc, :], x_t[:, cc, :], AF.Identity,
                                     bias=ab_t[:, cc, 1:2], scale=ab_t[:, cc, 0:1])

        # ===================== Q, K, Vp =====================
        # ones column for softmax denominators (evacs overwrite cols 0:C)
        nc.vector.memset(vpT_t[:], 1.0)
        with tc.tile_pool(name="qkv_ps", bufs=2, space="PSUM") as qkv_ps:
            # K (no bias needed: cancels in softmax)
            for ib in range(IB):
                i0 = ib * 512
                for oc in range(CC):
                    k_ps = qkv_ps.tile([128, 512], f32, tag="kq", name="k_ps")
                    for kc in range(CC):
                        nc.tensor.matmul(k_ps[:],
                                         lhsT=wkT_t[:, kc, oc * 128:(oc + 1) * 128],
                                         rhs=xn_t[:, kc, i0:i0 + 512],
                                         start=(kc == 0), stop=(kc == 1))
                    nc.vector.tensor_copy(k_t[:, oc, i0:i0 + 512], k_ps[:])
            # Vp (projected V, bias added per key; col C stays ones)
            for jc in range(JC):
                vp_ps = qkv_ps.tile([128, C], f32, tag="vp", name="vp_ps")
                for kc in range(CC):
                    nc.tensor.matmul(vp_ps[:],
                                     lhsT=xn_t[:, kc, jc * 128:(jc + 1) * 128],
                                     rhs=wvpT_t[:, kc, :],
                                     start=(kc == 0), stop=(kc == 1))
                nc.vector.tensor_add(vpT_t[:, jc, 0:C], vp_ps[:], bvpb_t[:])
            # Q (carries bq/16; scale 1/sqrt(C) folded on host)
            for ib in range(IB):
                i0 = ib * 512
                for oc in range(CC):
                    q_ps = qkv_ps.tile([128, 512], f32, tag="kq", name="q_ps")
                    for kc in range(CC):
                        nc.tensor.matmul(q_ps[:],
                                         lhsT=wqT_t[:, kc, oc * 128:(oc + 1) * 128],
                                         rhs=xn_t[:, kc, i0:i0 + 512],
                                         start=(kc == 0), stop=(kc == 1))
                    nc.vector.tensor_scalar_add(q_t[:, oc, i0:i0 + 512], q_ps[:],
                                                bq_t[:, oc, :])

        # ===================== Attention =====================
        # sT[j, i] = k^T q on 128-j x 512-i tiles; exp on ACT; the AV matmul
        # accumulates [i, c]+denominator over all j into PSUM.
        with tc.tile_pool(name="s_ps_p", bufs=2, space="PSUM") as s_ps_p, \
             tc.tile_pool(name="o2_ps_p", bufs=1, space="PSUM") as o2_ps_p, \
             tc.tile_pool(name="e_p", bufs=3) as e_p, \
             tc.tile_pool(name="nrm", bufs=4) as nrm_p:
            for ib in range(IB):
                i0 = ib * 512
                o2_ps = [o2_ps_p.tile([128, C + 1], f32, tag=f"o2_{u}",
                                      name=f"o2_ps{u}") for u in range(4)]
                for it in range(JC // 2):
                    s_ps = s_ps_p.tile([128, 1024], f32)
                    for jj in range(2):
                        jc = it * 2 + jj
                        for kc in range(CC):
                            nc.tensor.matmul(s_ps[:, jj * 512:(jj + 1) * 512],
                                             lhsT=k_t[:, kc, jc * 128:(jc + 1) * 128],
                                             rhs=q_t[:, kc, i0:i0 + 512],
                                             start=(kc == 0), stop=(kc == 1))
                    e_t = e_p.tile([128, 1024], bf16)
                    nc.scalar.activation(e_t[:], s_ps[:], AF.Exp)
                    for jj in range(2):
                        jc = it * 2 + jj
                        for u in range(4):
                            nc.tensor.matmul(
                                o2_ps[u][:],
                                lhsT=e_t[:, jj * 512 + u * 128: jj * 512 + (u + 1) * 128],
                                rhs=vpT_t[:, jc, :],
                                start=(it == 0 and jj == 0),
                                stop=(it == JC // 2 - 1 and jj == 1))
                for u in range(4):
                    rec_t = nrm_p.tile([128, 1], f32)
                    nc.vector.reciprocal(rec_t[:], o2_ps[u][:, C:C + 1])
                    nc.vector.tensor_scalar_mul(o2_t[:, ib * 4 + u, :],
                                                o2_ps[u][:, 0:C], rec_t[:])

        # ===================== Transpose + residual + store =====================
        with tc.tile_pool(name="tr_ps", bufs=3, space="PSUM") as tr_ps_p, \
             tc.tile_pool(name="res_p", bufs=3) as res_p:
            for cc in range(CC):
                for ig in range(IB):
                    t_ps = tr_ps_p.tile([128, 512], bf16)
                    for u in range(4):
                        ic = ig * 4 + u
                        nc.tensor.transpose(t_ps[:, u * 128:(u + 1) * 128],
                                            o2_t[:, ic, cc * 128:(cc + 1) * 128],
                                            ident_t[:])
                    res_t = res_p.tile([128, 512], f32)
                    nc.vector.tensor_add(res_t[:], x_t[:, cc, ig * 512:(ig + 1) * 512],
                                         t_ps[:])
                    nc.sync.dma_start(
                        out_d[cc * 128:(cc + 1) * 128, ig * 512:(ig + 1) * 512],
                        res_t[:])


_PROG = None


def _get_program():
    global _PROG
    if _PROG is None:
        nc = bacc.Bacc("TRN2", target_bir_lowering=False, debug=False,
                       num_devices=NCORES)
        _PROG = build_program(nc)
    return _PROG


def prep_in_maps(x, gn_gamma, gn_beta, wq, bq, wk, bk, wv, bv, wp, bp):
    """Host-side preprocessing: folds + per-core sharding."""
    x = np.asarray(x, np.float32)
    f64 = np.float64
    wq64, bq64 = np.asarray(wq, f64), np.asarray(bq, f64)
    wv64, bv64 = np.asarray(wv, f64), np.asarray(bv, f64)
    wp64, bp64 = np.asarray(wp, f64), np.asarray(bp, f64)
    scale = 1.0 / np.sqrt(C)

    wvp = wp64 @ wv64                    # [o, c]
    bvp = wp64 @ bv64 + bp64             # [o]

    gidx = np.arange(C) // GS
    gsum = (gidx[:, None] == np.arange(G)[None, :]).astype(np.float32)  # [C, G]
    gbc = gsum.T.copy()                                                  # [G, C]

    shared = {
        "wqT": np.ascontiguousarray((np.asarray(wq, f64).T * scale)).astype(BF16),
        "wkT": np.ascontiguousarray(np.asarray(wk, f64).T).astype(BF16),
        "wvpT": np.ascontiguousarray(wvp.T).astype(BF16),
        "bq": (bq64[:, None] * scale).astype(np.float32),
        "bvpb": np.tile(bvp[None, :], (128, 1)).astype(BF16),
        "gam": np.asarray(gn_gamma, np.float32)[:, None].copy(),
        "bet": np.asarray(gn_beta, np.float32)[:, None].copy(),
        "gsum": gsum,
        "gbc": np.ascontiguousarray(gbc),
        "ident": np.eye(128, dtype=np.float32).astype(BF16),
    }
    return [dict(shared, x=np.ascontiguousarray(x[i].reshape(C, HW)))
            for i in range(NCORES)]


def kernel(**inputs) -> np.ndarray:
    nc = _get_program()
    in_maps = prep_in_maps(**inputs)
    res = run_bass_kernel_spmd(nc, in_maps, core_ids=list(range(NCORES)))
    out = np.stack([np.asarray(res.results[i]["out"], np.float32).reshape(C, H, W)
                    for i in range(NCORES)])
    return out
